# revision 23
# baseline (speedup 1.0000x reference)
"""GQA attention (32 heads, 8 KV groups, rope, causal) on 8 TRN2 NeuronCores.

Sharding: tensor-parallel over KV groups — core g owns KV group g
(4 query heads + 1 kv head). Wq/Wk/Wv sharded column-wise, Wo row-wise;
each core produces a partial transposed output outT=[D,T] (bf16), summed
and transposed on the host.

v2 vs baseline:
  - all matmul operands in bf16 (same PE rate as f32r, half the DMA/SBUF)
  - output GEMM interleaved per chunk: out(c-1) runs between proj(c) and
    attn(c), hiding rope latency and removing the serial output tail
  - causal range restriction on diagonal tiles: score/exp/ctx/rowsum
    moving restricted to the valid query range
  - all 4 heads' softmax row-sums packed into one PSUM bank (partition
    offsets 0/32/64/96); reciprocal straight from PSUM, then gpsimd
    broadcast; ctx normalize fused as one DVE mul from PSUM
  - output written bf16 (host sums the 8 partials in f32)
"""
import math

import numpy as np

import concourse.bass as bass
import concourse.tile as tile
from concourse import bacc, mybir
from concourse.bass_utils import run_bass_kernel_spmd
from concourse.masks import make_identity

F32 = mybir.dt.float32
BF16 = mybir.dt.bfloat16

T = 2048          # tokens
D = 4096          # model dim
HD = 128          # head dim
NH = 4            # heads per core
DQ = NH * HD      # 512 q dims per core
TC = 512          # token chunk (psum free dim)
NCH = T // TC     # 4 chunks
KT = D // 128     # 32 contraction tiles
NET = D // 128    # 32 output-row tiles (of outT)
SCALE = 1.0 / math.sqrt(HD)
NCORES = 8


def build_nc():
    nc = bacc.Bacc("TRN2", target_bir_lowering=False, debug=False, num_devices=NCORES)
    # all big operands host-pre-shuffled to partition-major [128, k, m]
    # layouts so each load is one (or a few) fully-contiguous DMAs
    xT = nc.dram_tensor("xT", [128, KT, T], BF16, kind="ExternalInput").ap()
    wq = nc.dram_tensor("wq", [128, KT, DQ], BF16, kind="ExternalInput").ap()
    wk = nc.dram_tensor("wk", [128, KT, HD], BF16, kind="ExternalInput").ap()
    wv = nc.dram_tensor("wv", [128, KT, HD], BF16, kind="ExternalInput").ap()
    wo = nc.dram_tensor("wo", [128, NET, NH * 128], BF16, kind="ExternalInput").ap()
    cosT = nc.dram_tensor("cosT", [HD, T], F32, kind="ExternalInput").ap()
    sinT = nc.dram_tensor("sinT", [HD, T], F32, kind="ExternalInput").ap()
    ones = nc.dram_tensor("ones", [128, 2], BF16, kind="ExternalInput").ap()
    out = nc.dram_tensor("out", [D, T], BF16, kind="ExternalOutput").ap()

    with tile.TileContext(nc) as tc:
        _body(tc, out, xT, wq, wk, wv, wo, cosT, sinT, ones)
    nc.compile()
    return nc


def _body(tc, out, xT, wq, wk, wv, wo, cosT, sinT, ones):
    nc = tc.nc
    from contextlib import ExitStack

    with ExitStack() as ctx:
        const_pool = ctx.enter_context(tc.tile_pool(name="const", bufs=1))
        w_pool = ctx.enter_context(tc.tile_pool(name="wp", bufs=1))
        x_pool = ctx.enter_context(tc.tile_pool(name="xp", bufs=3))
        qt_pool = ctx.enter_context(tc.tile_pool(name="qtp", bufs=4))
        kt_pool = ctx.enter_context(tc.tile_pool(name="ktp", bufs=4))
        v_pool = ctx.enter_context(tc.tile_pool(name="vp", bufs=16))
        vt_pool = ctx.enter_context(tc.tile_pool(name="vtp", bufs=1))
        pt_pool = ctx.enter_context(tc.tile_pool(name="ptp", bufs=6))
        cx_pool = ctx.enter_context(tc.tile_pool(name="cxp", bufs=16))
        rope_pool = ctx.enter_context(tc.tile_pool(name="ropep", bufs=3))
        rb_pool = ctx.enter_context(tc.tile_pool(name="rbp", bufs=2))
        rc_pool = ctx.enter_context(tc.tile_pool(name="rcp", bufs=4))
        o_pool = ctx.enter_context(tc.tile_pool(name="op", bufs=4))
        cs_pool = ctx.enter_context(tc.tile_pool(name="csp", bufs=2))
        ps_pool = ctx.enter_context(tc.tile_pool(name="ps", bufs=8, space="PSUM"))

        # ---- constants (scalar-engine DGE so the sync queue starts on x) ----
        ones_sb = const_pool.tile([128, 2], BF16, tag="ones")
        ident_sb = const_pool.tile([128, 128], BF16, tag="ident")
        nc.scalar.dma_start(ones_sb[:], ones[:, :])
        make_identity(nc, ident_sb[:])

        # ---- resident weights (bf16); loaded inside chunk-0/1 k-loops so
        # the sync queue serves the first matmuls' inputs immediately ----
        wq_sb = w_pool.tile([128, KT, DQ], BF16, tag="wq")
        wk_sb = w_pool.tile([128, KT, HD], BF16, tag="wk")
        wv_sb = w_pool.tile([128, KT, HD], BF16, tag="wv")
        wo_sb = w_pool.tile([128, NET, NH * 128], BF16, tag="wo")

        kt_tiles = []      # kT chunk tiles [128, TC] (d x tokens), bf16
        v_tiles = []       # v j-tiles [128, 128] (tokens x d), bf16
        cx_tiles = {}      # (h, chunk) -> ctxT tile [128, TC], bf16

        def emit_output(c, et0=0, et1=NET):
            """outT[e, t-chunk c] = sum_h wo_h^T-tile @ ctxT_h(c)."""
            for Et in range(et0, et1):
                ps_o = ps_pool.tile([128, TC], F32, tag="ps",
                                    name=f"pso_{Et}_{c}")
                for h in range(NH):
                    nc.tensor.matmul(
                        ps_o[:],
                        wo_sb[:, Et:Et + 1, h * 128:(h + 1) * 128],
                        cx_tiles[(h, c)][:],
                        start=h == 0, stop=h == NH - 1,
                    )
                ot = o_pool.tile([128, TC], BF16, tag="o", name=f"o_{Et}_{c}")
                if Et % 2 == 0:
                    nc.vector.tensor_copy(ot[:], ps_o[:])
                else:
                    nc.scalar.copy(ot[:], ps_o[:])
                # out DMAs on sync (x needs only 8 triggers/chunk); gpsimd
                # stays clear for the causal selects + broadcasts
                nc.sync.dma_start(
                    out[Et * 128:(Et + 1) * 128, c * TC:(c + 1) * TC], ot[:]
                )

        for c in range(NCH):
            # ================= projections for token chunk c =================
            ps_q = [ps_pool.tile([128, TC], F32, tag="ps", name=f"psq{h}_{c}")
                    for h in range(NH)]
            ps_k = ps_pool.tile([128, TC], F32, tag="ps", name=f"psk_{c}")
            ps_v = ps_pool.tile([128, TC], F32, tag="ps", name=f"psv_{c}")
            for k in range(KT):
                g, kk = k // 4, k % 4
                if kk == 0:
                    # weights stream on the scalar DGE queue, x on sync —
                    # separate rings so neither stalls the other
                    if c == 0:
                        nc.gpsimd.dma_start(wq_sb[:, 4 * g:4 * g + 4, :],
                                            wq[:, 4 * g:4 * g + 4, :])
                        nc.gpsimd.dma_start(wk_sb[:, 4 * g:4 * g + 4, :],
                                            wk[:, 4 * g:4 * g + 4, :])
                        nc.gpsimd.dma_start(wv_sb[:, 4 * g:4 * g + 4, :],
                                            wv[:, 4 * g:4 * g + 4, :])
                    if c == 1:
                        nc.sync.dma_start(wo_sb[:, 4 * g:4 * g + 4, :],
                                          wo[:, 4 * g:4 * g + 4, :])
                    xt4 = x_pool.tile([128, 4, TC], BF16, tag="x",
                                      name=f"x_{c}_{g}")
                    nc.sync.dma_start(
                        xt4[:], xT[:, 4 * g:4 * g + 4, c * TC:(c + 1) * TC]
                    )
                xt = xt4[:, kk:kk + 1, :]
                first, last = k == 0, k == KT - 1
                for h in range(NH):
                    nc.tensor.matmul(
                        ps_q[h][:],
                        wq_sb[:, k:k + 1, h * HD:(h + 1) * HD],
                        xt,
                        start=first, stop=last,
                    )
                nc.tensor.matmul(
                    ps_k[:], wk_sb[:, k:k + 1, :], xt,
                    start=first, stop=last,
                )
                nc.tensor.matmul(
                    ps_v[:], wv_sb[:, k:k + 1, :], xt,
                    start=first, stop=last,
                )

            cs_t = cs_pool.tile([HD, TC], F32, tag="cos", name=f"cos_{c}")
            sn_t = cs_pool.tile([HD, TC], F32, tag="sin", name=f"sin_{c}")
            nc.scalar.dma_start(cs_t[:], cosT[:, c * TC:(c + 1) * TC])
            nc.scalar.dma_start(sn_t[:], sinT[:, c * TC:(c + 1) * TC])
            cs = cs_t[:, :]
            sn = sn_t[:, :]

            def rope(ps, dst_pool, tag, nm, stage_eng):
                # one copy frees the PSUM bank (vs 3 direct reads); sn holds
                # sign-folded sin with halves pre-swapped so both rotate-half
                # muls read SBUF operands at matching base partitions
                st = rope_pool.tile([128, TC], F32, tag="st", name=f"st{nm}")
                if stage_eng == "v":
                    nc.vector.tensor_copy(st[:], ps[:])
                else:
                    nc.scalar.copy(st[:], ps[:])
                t1 = rope_pool.tile([128, TC], F32, tag="t1", name=f"r1{nm}")
                t2 = rope_pool.tile([128, TC], F32, tag="t2", name=f"r2{nm}")
                nc.vector.tensor_mul(t2[0:64, :], st[64:128, :], sn[64:128, :])
                nc.vector.tensor_mul(t2[64:128, :], st[0:64, :], sn[0:64, :])
                nc.vector.tensor_mul(t1[:], st[:], cs)
                d = dst_pool.tile([128, TC], BF16, tag=tag, name=nm)
                nc.vector.tensor_add(d[:], t1[:], t2[:])
                return d

            # k first: attention needs kt before S matmuls
            kt = rope(ps_k, kt_pool, "kt", f"kt_{c}", "v")
            kt_tiles.append(kt)

            # v: psum -> sbuf (bf16), PE-transposed to tokens-major later
            vt = vt_pool.tile([128, TC], BF16, tag="vt", name=f"vt_{c}")
            nc.scalar.copy(vt[:], ps_v[:])

            q_chunk = [rope(ps_q[h], qt_pool, "qt", f"qt_{c}_{h}",
                            "v" if h % 2 else "s")
                       for h in range(NH)]

            # ======= output stage (first half) for the previous chunk: PE
            # stays busy while DVE ropes chunk c; only half the PSUM-bank
            # demand while the proj banks drain =======
            if c >= 1:
                emit_output(c - 1, 0, NET // 2)

            # v transpose: each [128,128] to tokens-major via PE (bf16)
            for jj in range(TC // 128):
                ps_t = ps_pool.tile([128, 128], BF16, tag="ps",
                                    name=f"pst_{c}_{jj}")
                nc.tensor.transpose(ps_t[:], vt[:, jj * 128:(jj + 1) * 128],
                                    ident_sb[:])
                vsb = v_pool.tile([128, 128], BF16, tag="v", name=f"v_{c}_{jj}")
                nc.vector.tensor_copy(vsb[:], ps_t[:])
                v_tiles.append(vsb)

            # ========== attention for i-chunk I = c, two heads at a time =====
            I = c
            nj = 4 * I + 4
            deferred = []
            for hp in range(NH // 2):
                hs = [2 * hp, 2 * hp + 1]
                # the pair's row-sums share one PSUM bank at partition
                # offsets 0 and 64 (hw allows base partition 0/32/64 only)
                ps_sum = ps_pool.tile([128, TC], F32, tag="ps",
                                      name=f"pssum_{I}_{hp}")
                ps_ctx = {h: ps_pool.tile([128, TC], F32, tag="ps",
                                          name=f"psctx_{I}_{h}") for h in hs}

                def scores(J):
                    # causal restriction: key tile J only feeds queries
                    # i >= lo (diagonal tiles); off-diagonal lo = 0
                    q = J - 4 * I
                    lo = max(0, q) * 128
                    pts = {}
                    for h in hs:  # kT_J stationary shared across the pair
                        ps_s = ps_pool.tile([128, TC], F32, tag="ps",
                                            name=f"pss_{I}_{h}_{J}")
                        nc.tensor.matmul(
                            ps_s[:, lo:TC],
                            kt_tiles[J // 4][:, (J % 4) * 128:(J % 4 + 1) * 128],
                            q_chunk[h][:, lo:TC],
                            start=True, stop=True,
                        )
                        pt = pt_pool.tile([128, TC], BF16, tag="pt",
                                          name=f"pt_{I}_{h}_{J}")
                        nc.scalar.activation(
                            pt[:, lo:TC], ps_s[:, lo:TC],
                            mybir.ActivationFunctionType.Exp,
                            scale=SCALE,
                        )
                        if q >= 0:
                            # causal zeroing post-exp on the idle gpsimd:
                            # keep pt[j, lo+y] iff y >= j (lo == q*128)
                            nc.gpsimd.affine_select(
                                pt[:, lo:TC], pt[:, lo:TC],
                                pattern=[[1, TC - lo]],
                                compare_op=mybir.AluOpType.is_ge,
                                fill=0.0,
                                base=0,
                                channel_multiplier=-1,
                            )
                        pts[h] = pt
                    return pts

                def ctx_sum(J, pts):
                    q = J - 4 * I
                    lo = max(0, q) * 128
                    first, last = J == 0, J == nj - 1
                    for h in hs:  # v_J stationary shared across the pair
                        nc.tensor.matmul(
                            ps_ctx[h][:, lo:TC], v_tiles[J][:],
                            pts[h][:, lo:TC],
                            start=first, stop=last, skip_group_check=True,
                        )
                    for hi, h in enumerate(hs):  # ones stationary (trivial ld)
                        nc.tensor.matmul(
                            ps_sum[64 * hi:64 * hi + 2, lo:TC], ones_sb[:],
                            pts[h][:, lo:TC],
                            start=first, stop=last, skip_group_check=True,
                        )

                # software-pipelined: scores/exp for J+1 issue before ctx/sum
                # of J so the PE never waits on the exp -> mask chain
                prev = None
                for J in range(nj + 1):
                    cur = scores(J) if J < nj else None
                    if prev is not None:
                        ctx_sum(J - 1, prev)
                    prev = cur

                # drain PSUM fast (one DVE + one ACT copy of the ctx banks,
                # tiny ACT copies of the sum rows); the recip/broadcast/
                # normalize chain is deferred past both pairs' J-loops so
                # pair-1 selects never queue behind pair-0 broadcasts
                for hi, h in enumerate(hs):
                    cu = cx_pool.tile([128, TC], BF16, tag="cxu",
                                      name=f"cxu_{I}_{h}")
                    if hi == 0:
                        nc.vector.tensor_copy(cu[:], ps_ctx[h][:])
                    else:
                        nc.scalar.copy(cu[:], ps_ctx[h][:])
                    srow = rc_pool.tile([1, TC], F32, tag="srow",
                                        name=f"sr_{I}_{h}")
                    nc.scalar.copy(srow[:], ps_sum[64 * hi:64 * hi + 1, :])
                    deferred.append((h, cu, srow))

            for h, cu, srow in deferred:
                rcp = rc_pool.tile([1, TC], F32, tag="recip",
                                   name=f"rc_{I}_{h}")
                nc.vector.reciprocal_approx_fast(rcp[:], srow[:])
                rb = rb_pool.tile([128, TC], F32, tag="rb", name=f"rb_{I}_{h}")
                nc.gpsimd.partition_broadcast(rb[:], rcp[:])
                cxt = cx_pool.tile([128, TC], BF16, tag="cx",
                                   name=f"cx_{I}_{h}")
                nc.vector.tensor_mul(cxt[:], cu[:], rb[:])
                cx_tiles[(h, I)] = cxt

            # second half of the previous chunk's output: all banks free now
            if c >= 1:
                emit_output(c - 1, NET // 2, NET)

        emit_output(NCH - 1)


# ---------------------------------------------------------------------------
# host side
# ---------------------------------------------------------------------------
_NC_CACHE = None


def _get_nc():
    global _NC_CACHE
    if _NC_CACHE is None:
        _NC_CACHE = build_nc()
    return _NC_CACHE


def make_in_maps(x, Wq, Wk, Wv, Wo, cos, sin):
    import ml_dtypes
    bf16 = ml_dtypes.bfloat16

    x = np.asarray(x, dtype=np.float32)
    # [128, KT, T]: [p, k, t] = x[t, k*128+p]
    xT = np.ascontiguousarray(
        x.reshape(T, KT, 128).transpose(2, 1, 0).astype(bf16)
    )
    cosT = np.ascontiguousarray(np.asarray(cos, np.float32)[:T].T)
    sin_t = np.asarray(sin, np.float32)[:T]          # [T, 128]
    sinT = sin_t.T.copy()                            # [128, T]
    sinT[:64] *= -1.0                                # fold rotate-half sign
    # pre-swap halves: kernel reads sn[p] where the math wants sn_f[p^64]
    sinT = np.ascontiguousarray(np.concatenate([sinT[64:], sinT[:64]], axis=0))

    ones = np.ones((128, 2), bf16)

    Wq = np.asarray(Wq, np.float32)
    Wk = np.asarray(Wk, np.float32)
    Wv = np.asarray(Wv, np.float32)
    Wo = np.asarray(Wo, np.float32)
    def shuf(w):
        # [Dk*128, M] -> [128, Dk, M] partition-major contiguous
        kt, m_ = w.shape[0] // 128, w.shape[1]
        return np.ascontiguousarray(
            w.reshape(kt, 128, m_).transpose(1, 0, 2).astype(bf16)
        )

    # per core: wo rows [g*DQ:(g+1)*DQ] shuffled to [dh, Et, (h, e)]
    woP = np.empty((NCORES, 128, NET, NH * 128), bf16)
    for g in range(NCORES):
        w = Wo[g * DQ:(g + 1) * DQ, :]                    # [512, 4096]
        w4 = w.reshape(NH, HD, NET, 128).transpose(1, 2, 0, 3)  # [dh, Et, h, e]
        woP[g] = w4.reshape(128, NET, NH * 128).astype(bf16)
    in_maps = []
    for g in range(NCORES):
        in_maps.append({
            "xT": xT,
            "wq": shuf(Wq[:, g * DQ:(g + 1) * DQ]),
            "wk": shuf(Wk[:, g * HD:(g + 1) * HD]),
            "wv": shuf(Wv[:, g * HD:(g + 1) * HD]),
            "wo": woP[g],
            "cosT": cosT,
            "sinT": sinT,
            "ones": ones,
        })
    return in_maps


def kernel(x, Wq, Wk, Wv, Wo, cos, sin):
    nc = _get_nc()
    in_maps = make_in_maps(x, Wq, Wk, Wv, Wo, cos, sin)
    res = run_bass_kernel_spmd(nc, in_maps, core_ids=list(range(NCORES)))
    acc = np.zeros((D, T), np.float32)
    for c in range(NCORES):
        acc += res.results[c]["out"].astype(np.float32)
    return np.ascontiguousarray(acc.T).reshape(1, T, D)
